# revision 6
# baseline (speedup 1.0000x reference)
"""Trainium2 Bass kernel for nn_MixedConvWithReLU — 1-D Winograd F(2,3), v2.

v2 vs baseline: same Winograd F(2,3) conv structure, but the elementwise side
is redesigned around the DVE/ScalarE cost model ((58|120|172 + FD/Accel)/f):

- stage-1 input transform: 3 large contiguous fp16 ops (za full-width 2x mode,
  zb/zc strided even-only) instead of 8 strided 1x ops.
- RG 8 -> 14 rows per PSUM tile (FD 224 -> 392) halves instruction count.
- branch 1 (4-bit) and branch 2 (16-bit) epilogues run in fp16 (2-4x DVE
  modes) off ScalarE PSUM->fp16 copies with the BN scale folded into the copy.
- branch 1's round-to-15-levels uses fp16 RNE at a +1536 magic offset applied
  by the last scalar_tensor_tensor, then (x-1536)*w1/15 strips the offset
  before any further fp16 write (offset in fp16 would cost 1/32 ulp noise).
- branch 0 (binary) stays fp32 end-to-end: its threshold flip error scales as
  sqrt(conv error), fp16 there would blow the 2e-2 budget.

Predicted end-to-end rel err 7.5e-3 (numpy sim) vs the 2e-2 gate.
"""
import numpy as np
import concourse.bacc as bacc
import concourse.tile as tile
import concourse.mybir as mybir
from concourse.bass_utils import run_bass_kernel_spmd

F32 = mybir.dt.float32
F16 = mybir.dt.float16
I32 = mybir.dt.int32
AF = mybir.ActivationFunctionType
ALU = mybir.AluOpType

N_CORES = 8
B, CIN, COUT, H, W, K = 32, 256, 256, 56, 56, 3
B_PER = B // N_CORES          # 4 images per core
RG = 14                       # rows per psum tile
N_RB = H // RG                # 4 row-blocks
HP = H + 2                    # padded 58
NQ = W // 2                   # 28 output column pairs
EPS = 1e-5

_cache = {}


def _quant_int(Wb, bits):
    Wb = Wb.astype(np.float32)
    levels = 2 ** (bits - 1) - 1
    step = np.float32(np.max(np.abs(Wb)) / np.float32(levels))
    return np.round(Wb / step).astype(np.float32), np.float64(step)


def _build(loop=True):
    nc = bacc.Bacc(trn_type="TRN2", debug=False)
    xr = nc.dram_tensor("xr", [B_PER, CIN, H, W], F16, kind="ExternalInput").ap()
    wr = nc.dram_tensor("wr", [128, 2 * 3 * 4 * 6 * 128], F16,
                        kind="ExternalInput").ap()
    cst = nc.dram_tensor("cst", [128, 16], F32, kind="ExternalInput").ap()
    iters = nc.dram_tensor("iters", [1, 1], I32, kind="ExternalInput").ap()
    out = nc.dram_tensor("out", [B_PER, COUT, H, W], F32, kind="ExternalOutput").ap()

    with tile.TileContext(nc) as tc:
        with (
            tc.tile_pool(name="fix", bufs=1) as fix,
            tc.tile_pool(name="ps", bufs=8, space="PSUM") as ps,
            tc.tile_pool(name="stage", bufs=2) as stage,
        ):
            wsb = fix.tile([128, 2, 3, 4, 6, 128], F16, tag="wsb")
            cst_t = fix.tile([128, 16], F32, tag="cst")
            xp = [fix.tile([128, 2, HP, HP], F16, tag=f"xp{s}", name=f"xp{s}")
                  for s in range(2)]
            za = [fix.tile([128, 2, HP, 56], F16, tag=f"za{s}", name=f"za{s}")
                  for s in range(2)]
            zb = [fix.tile([128, 2, HP, NQ], F16, tag=f"zb{s}", name=f"zb{s}")
                  for s in range(2)]
            zc = [fix.tile([128, 2, HP, NQ], F16, tag=f"zc{s}", name=f"zc{s}")
                  for s in range(2)]

            nc.sync.dma_start(
                out=wsb[:].rearrange("p h kh pt b m -> p (h kh pt b m)"), in_=wr)
            nc.sync.dma_start(out=cst_t[:], in_=cst)

            if loop:
                tmp = nc.alloc_registers("iters_reg", mybir.ALL_ENGINES)
                nc.regs_load(tmp, iters[0:1, 0:1])
                n_it = nc.snap(tmp, donate=True, min_val=1, max_val=1000000)

            for s in range(2):
                nc.vector.memset(xp[s][:], 0.0)

            from contextlib import nullcontext
            with (tc.For_i(0, n_it, 1) if loop else nullcontext()):
                if loop:
                    nc.gpsimd.nop()
                for img in range(B_PER):
                    s = img % 2
                    for h in range(2):
                        # gpsimd queue: don't serialize behind scalar/vector
                        nc.gpsimd.dma_start(
                            out=xp[s][:, h, 1:H + 1, 1:W + 1],
                            in_=xr[img, 128 * h:128 * (h + 1), :, :])
                    # stage-1: za[c]=x[c]-x[c+2] (pt0@even, pt3@odd cols),
                    # zb=d1+d2 (pt1), zc=d1-d2 (= -true z2 -> m2 negated)
                    xq = xp[s][:].rearrange("p h r (q t) -> p h r q t", t=2)
                    d_odd = xq[:, :, :, 0:NQ, 1]       # cols 1,3,..,55
                    d_ev2 = xq[:, :, :, 1:NQ + 1, 0]   # cols 2,4,..,56
                    nc.vector.tensor_tensor(
                        out=za[s][:], in0=xp[s][:, :, :, 0:56],
                        in1=xp[s][:, :, :, 2:58], op=ALU.subtract)
                    # strided 1x ops: offload to the otherwise-idle gpsimd
                    nc.gpsimd.tensor_tensor(
                        out=zb[s][:], in0=d_odd, in1=d_ev2, op=ALU.add)
                    nc.gpsimd.tensor_tensor(
                        out=zc[s][:], in0=d_odd, in1=d_ev2, op=ALU.subtract)
                    zaq = za[s][:].rearrange("p h r (q t) -> p h r q t", t=2)

                    for j in range(2):
                        s0s = {}
                        tEs = {}
                        for br in range(3):
                            blk = 2 * br + j
                            for rb in range(N_RB):
                                r0 = RG * rb
                                m = [ps.tile([128, RG, NQ], F32, tag="ps",
                                             name=f"m{pt}") for pt in range(4)]
                                for pt in range(4):
                                    n = 0
                                    for h in range(2):
                                        for kh in range(3):
                                            rs = slice(r0 + kh, r0 + kh + RG)
                                            if pt == 0:
                                                rhs = zaq[:, h, rs, :, 0]
                                            elif pt == 1:
                                                rhs = zb[s][:, h, rs, :]
                                            elif pt == 2:
                                                rhs = zc[s][:, h, rs, :]
                                            else:
                                                rhs = zaq[:, h, rs, :, 1]
                                            nc.tensor.matmul(
                                                out=m[pt][:],
                                                lhsT=wsb[:, h, kh, pt, blk, :],
                                                rhs=rhs,
                                                start=(n == 0), stop=(n == 5))
                                            n += 1
                                # NOTE: m[2] holds the NEGATED true m2
                                if br == 0:
                                    # fp32 path (binary branch, flip-sensitive)
                                    m1s = stage.tile([128, RG, NQ], F32,
                                                     tag="m1s", name="m1s", bufs=2)
                                    nc.scalar.activation(
                                        out=m1s[:], in_=m[1][:], func=AF.Identity,
                                        bias=0.0, scale=1.0)
                                    t01 = stage.tile([128, RG, NQ], F32,
                                                     tag="t01", name="t01", bufs=2)
                                    nc.vector.tensor_tensor(
                                        out=t01[:], in0=m[0][:], in1=m1s[:],
                                        op=ALU.add)
                                    t13 = stage.tile([128, RG, NQ], F32,
                                                     tag="t13", name="t13", bufs=2)
                                    nc.vector.tensor_tensor(
                                        out=t13[:], in0=m1s[:], in1=m[3][:],
                                        op=ALU.subtract)
                                    y0p = stage.tile([128, 2, RG, NQ], F32,
                                                     tag="y0p", name="y0p", bufs=2)
                                    nc.vector.tensor_tensor(
                                        out=y0p[:, 0], in0=t01[:], in1=m[2][:],
                                        op=ALU.subtract)
                                    nc.vector.tensor_tensor(
                                        out=y0p[:, 1], in0=t13[:], in1=m[2][:],
                                        op=ALU.add)
                                    # s0' = (y0 > c0) * (15*w0/w1): pre-divided
                                    # by w1/15 so the final merge rescales once
                                    s0p = stage.tile([128, 2, RG, NQ], F16,
                                                     tag="s0p", name="s0p", bufs=3)
                                    nc.vector.tensor_scalar(
                                        out=s0p[:], in0=y0p[:],
                                        scalar1=cst_t[:, 0 + j:1 + j],
                                        scalar2=cst_t[:, 13:14],
                                        op0=ALU.is_gt, op1=ALU.mult)
                                    s0s[rb] = s0p
                                elif br == 1:
                                    # fp16 path, x15*s1 folded into the copies,
                                    # +1536 magic round at the ye/yo STT write
                                    ms = []
                                    for i in range(4):
                                        mt = stage.tile([128, RG, NQ], F16,
                                                        tag=f"msa{i}",
                                                        name=f"msa{i}", bufs=2)
                                        nc.scalar.activation(
                                            out=mt[:], in_=m[i][:],
                                            func=AF.Identity, bias=0.0,
                                            scale=cst_t[:, 2 + j:3 + j])
                                        ms.append(mt)
                                    t01 = stage.tile([128, RG, NQ], F16,
                                                     tag="t01b", name="t01b", bufs=2)
                                    nc.vector.tensor_tensor(
                                        out=t01[:], in0=ms[0][:], in1=ms[1][:],
                                        op=ALU.add)
                                    t13 = stage.tile([128, RG, NQ], F16,
                                                     tag="t13b", name="t13b", bufs=2)
                                    nc.vector.tensor_tensor(
                                        out=t13[:], in0=ms[1][:], in1=ms[3][:],
                                        op=ALU.subtract)
                                    y1p = stage.tile([128, 2, RG, NQ], F16,
                                                     tag="y1p", name="y1p", bufs=2)
                                    nc.vector.scalar_tensor_tensor(
                                        out=y1p[:, 0], in0=t01[:],
                                        scalar=cst_t[:, 4 + j:5 + j],
                                        in1=ms[2][:], op0=ALU.add,
                                        op1=ALU.subtract)
                                    nc.vector.scalar_tensor_tensor(
                                        out=y1p[:, 1], in0=t13[:],
                                        scalar=cst_t[:, 4 + j:5 + j],
                                        in1=ms[2][:], op0=ALU.add, op1=ALU.add)
                                    kp = stage.tile([128, 2, RG, NQ], F16,
                                                    tag="kp", name="kp", bufs=2)
                                    nc.vector.tensor_scalar(
                                        out=kp[:], in0=y1p[:],
                                        scalar1=1551.0, scalar2=1536.0,
                                        op0=ALU.min, op1=ALU.max)
                                    s0p = s0s.pop(rb)
                                    tp = stage.tile([128, 2, RG, NQ], F16,
                                                    tag="tp", name="tp", bufs=3)
                                    nc.vector.scalar_tensor_tensor(
                                        out=tp[:], in0=kp[:], scalar=1536.0,
                                        in1=s0p[:], op0=ALU.subtract,
                                        op1=ALU.add)
                                    tEs[rb] = tp
                                else:
                                    # fp16 path, inv2*w2 folded into copies
                                    ms = []
                                    for i in range(4):
                                        mt = stage.tile([128, RG, NQ], F16,
                                                        tag=f"msb{i}",
                                                        name=f"msb{i}", bufs=2)
                                        nc.scalar.activation(
                                            out=mt[:], in_=m[i][:],
                                            func=AF.Identity, bias=0.0,
                                            scale=cst_t[:, 6 + j:7 + j])
                                        ms.append(mt)
                                    t01 = stage.tile([128, RG, NQ], F16,
                                                     tag="t01c", name="t01c", bufs=2)
                                    nc.vector.tensor_tensor(
                                        out=t01[:], in0=ms[0][:], in1=ms[1][:],
                                        op=ALU.add)
                                    t13 = stage.tile([128, RG, NQ], F16,
                                                     tag="t13c", name="t13c", bufs=2)
                                    nc.vector.tensor_tensor(
                                        out=t13[:], in0=ms[1][:], in1=ms[3][:],
                                        op=ALU.subtract)
                                    y2p = stage.tile([128, 2, RG, NQ], F16,
                                                     tag="y2p", name="y2p", bufs=2)
                                    nc.vector.scalar_tensor_tensor(
                                        out=y2p[:, 0], in0=t01[:],
                                        scalar=cst_t[:, 8 + j:9 + j],
                                        in1=ms[2][:], op0=ALU.add,
                                        op1=ALU.subtract)
                                    nc.vector.scalar_tensor_tensor(
                                        out=y2p[:, 1], in0=t13[:],
                                        scalar=cst_t[:, 8 + j:9 + j],
                                        in1=ms[2][:], op0=ALU.add, op1=ALU.add)
                                    vp = stage.tile([128, 2, RG, NQ], F16,
                                                    tag="vp", name="vp", bufs=2)
                                    nc.vector.tensor_scalar(
                                        out=vp[:], in0=y2p[:],
                                        scalar1=cst_t[:, 12:13], scalar2=0.0,
                                        op0=ALU.min, op1=ALU.max)
                                    o = stage.tile([128, RG, W], F32,
                                                   tag="o", name="o", bufs=4)
                                    ov = o[:].rearrange("p r (q t) -> p t r q",
                                                        t=2)
                                    tp = tEs.pop(rb)
                                    nc.vector.scalar_tensor_tensor(
                                        out=ov, in0=tp[:],
                                        scalar=cst_t[:, 11:12], in1=vp[:],
                                        op0=ALU.mult, op1=ALU.add)
                                    nc.sync.dma_start(
                                        out=out[img, 128 * j:128 * (j + 1),
                                                r0:r0 + RG, :],
                                        in_=o[:])
                if loop:
                    nc.gpsimd.nop()

    nc.compile()
    return nc


def _prepare(x, Wt, bn_gamma, bn_beta, bn_mean, bn_var, alphas):
    x = np.ascontiguousarray(x, np.float32)
    Wt = np.asarray(Wt, np.float32)
    a64 = np.asarray(alphas, np.float64)
    e = np.exp(a64 - a64.max())
    wsoft = (e / e.sum()).astype(np.float64)
    w0, w1, w2 = wsoft

    inv = (np.asarray(bn_gamma, np.float64)
           / np.sqrt(np.asarray(bn_var, np.float64) + EPS))
    bias = (np.asarray(bn_beta, np.float64)
            - np.asarray(bn_mean, np.float64) * inv)

    scale0 = np.float64(np.mean(np.abs(Wt[0]), dtype=np.float32))
    Wdev = [np.sign(Wt[0]).astype(np.float64), None, None]
    k1, step1 = _quant_int(Wt[1], 4)
    Wdev[1] = k1.astype(np.float64)
    k2, step2 = _quant_int(Wt[2], 16)
    Wdev[2] = k2.astype(np.float64) * step2

    # GW points per branch: [4][Cout, Cin, 3(kh)]
    Whost = np.empty((128, 2, 3, 4, 6, 128), np.float16)
    for i in range(3):
        w = Wdev[i]                                   # [Cout, Cin, 3, 3] f64
        g = [w[:, :, :, 0],
             (w[:, :, :, 0] + w[:, :, :, 1] + w[:, :, :, 2]) / 2,
             (w[:, :, :, 0] - w[:, :, :, 1] + w[:, :, :, 2]) / 2,
             w[:, :, :, 2]]
        for j in range(2):
            blk = 2 * i + j
            for pt in range(4):
                sub = g[pt][128 * j:128 * (j + 1), :, :]   # [128m, 256, 3]
                for h in range(2):
                    # -> [cin_p, kh, cout_m]
                    Whost[:, h, :, pt, blk, :] = \
                        sub[:, 128 * h:128 * (h + 1), :].transpose(1, 2, 0) \
                        .astype(np.float16)
    wr = Whost.reshape(128, -1)

    cst = np.zeros((128, 16), np.float32)
    for j in range(2):
        sl = slice(128 * j, 128 * (j + 1))
        cst[:, 0 + j] = ((0.5 - bias[0][sl]) / (inv[0][sl] * scale0)) \
            .astype(np.float32)
        cst[:, 2 + j] = (15.0 * step1 * inv[1][sl]).astype(np.float32)
        cst[:, 4 + j] = (15.0 * bias[1][sl] + 1536.0).astype(np.float32)
        cst[:, 6 + j] = (inv[2][sl] * w2).astype(np.float32)
        cst[:, 8 + j] = (w2 * bias[2][sl]).astype(np.float32)
    cst[:, 10] = np.float32(w0)
    cst[:, 11] = np.float32(w1 / 15.0)
    cst[:, 12] = np.float32(w2)
    cst[:, 13] = np.float32(15.0 * w0 / w1)

    x16 = x.astype(np.float16)
    return x16, wr, cst


def kernel(x, W, bn_gamma, bn_beta, bn_mean, bn_var, alphas, _iters=1):
    loop = _iters > 1
    key = ("v3", loop)
    if key not in _cache:
        _cache[key] = _build(loop=loop)
    nc = _cache[key]
    x16, wr, cst = _prepare(x, W, bn_gamma, bn_beta, bn_mean, bn_var, alphas)

    it = np.array([[_iters]], np.int32)
    in_maps = [
        {"xr": x16[B_PER * c:B_PER * (c + 1)], "wr": wr, "cst": cst, "iters": it}
        for c in range(N_CORES)
    ]
    res = run_bass_kernel_spmd(nc, in_maps, list(range(N_CORES)))
    outs = [res.results[c]["out"] for c in range(N_CORES)]
    return np.concatenate(outs, axis=0)
